# revision 5
# baseline (speedup 1.0000x reference)
"""Trainium2 Bass kernel for nn_Model_39676907882504.

Math: qk = (q @ k^T)/8 has shape [1,2048,1,1]; after the transposes it is
[2048,1,1,1], and softmax over the trailing size-1 axis is exactly 1.0
regardless of qk (exp(x-max)/sum == 1/1 bit-exactly). The final matmul
[S,Q,B,Q] @ [B,S,Q,D] with attn_weight == 1 therefore reduces to
broadcasting `value` across a new leading dim:

    output[i, j, 0, :] = value[0, j, 0, :]   for all i in [0, 2048)

i.e. a 512KB -> 1GiB broadcast copy.  Pure memory-regime kernel.

Sharding (per the hint): leading output dim (2048 rows) split across the
8 cores, 256 rows/core (= 4096 sub-rows of 32KB; sub-row d holds value
chunk d%16); value replicated.

Trace-derived facts this kernel is built around:

  * A dma_start's descriptors are split over SDMA engines contiguously
    along the OUTER AP dim, using the largest divisor <= 16 (128 -> 16
    engines x 8, 120 -> 15 engines x 8, 8 -> engines 0-7 x 1).
  * Each engine moves a 32KB descriptor in ~1216ns (its ~27 GB/s line
    rate); the kernel is engine-bound at ~435 GB/s aggregate.
  * SDMA engine 15 intermittently suffers ~2us stalls every ~8
    descriptors (~20% capacity loss, present in most runs, absent in
    some) — so engine 15 gets a reduced share sized for its degraded
    rate: 16-way stores cover 25 of the 32 c-reps; the other 7 go via a
    15-way [120, 7] store (engines 0-14) plus an [8, 7] tail (engines
    0-7) covering partitions 120-127.
  * Few instructions: per-instruction completion work also lands on
    engine 15 (~2.7us each), so the whole kernel is 2 loads + 4 stores.

SBUF tile: [128, 8192], partition q = chunk q%16 (8 copies of value,
host-pre-tiled).  Stores re-read it via a stride-0 middle dim; the DRAM
side is out.rearrange("(c q) e -> q c e"): descriptor (q, c) lands at
sub-row q + 128c, whose required content is chunk q%16.

Engine budget: engines 0-7 ~272 descs, 8-14 ~264, engine 15 200 descs
(sick: 200 x ~1.55us = 310us < 272 x 1.216us = 331us) -> ~331us span
+ ~13us fixed NEFF entry/exit, robust to engine 15's state.
"""

import sys

for _p in ("/opt/trn_rl_repo",):
    if _p not in sys.path:
        sys.path.insert(0, _p)

import numpy as np

import concourse.bass as bass
import concourse.mybir as mybir
from concourse.bass_utils import run_bass_kernel_spmd

S = 2048
D = 64
N_CORES = 8
ROWS_PER_CORE = S // N_CORES          # 256 output rows/core, 512KB each
F = 8192                              # f32 per 32KB chunk; value = 16 chunks
SUBROWS = ROWS_PER_CORE * 16          # 4096 32KB sub-rows per shard
NREP = SUBROWS // 128                 # 32 broadcast reps of the 128-part tile

C16A = 13                             # 16-way store c-reps on queue A
C16B = 12                             # 16-way store c-reps on queue B
C15 = NREP - C16A - C16B              # 7 reps routed around engine 15

TRACE = False          # test.py flips this to profile
TRACE_KWARGS = {}
LAST_RESULT = None     # BassKernelResults of the last run (for test.py)


def build_program():
    nc = bass.Bass()
    val = nc.declare_dram_parameter("value", [128, F], mybir.dt.float32,
                                    isOutput=False)
    out = nc.declare_dram_parameter("out", [SUBROWS, F], mybir.dt.float32,
                                    isOutput=True)

    vtile = nc.alloc_sbuf_tensor("vtile", [128, F], mybir.dt.float32)

    # [q, c, e]: sub-row q + 128*c <- vtile partition q (chunk q%16).
    out_qce = out[:, :].rearrange("(c q) e -> q c e", q=128)

    def in_bcast(q0, q1, reps):
        return vtile[q0:q1, :].unsqueeze(1).broadcast_to((q1 - q0, reps, F))

    with nc.Block() as block, \
         nc.semaphore("lsem") as lsem, \
         nc.semaphore("s1") as s1, \
         nc.semaphore("s2") as s2:

        @block.sync
        def _(sync):
            # 15-way + 8-way loads keep engine 15 out of the load phase.
            sync.dma_start(out=vtile[0:120, :],
                           in_=val[0:120, :]).then_inc(lsem, 16)
            sync.dma_start(out=vtile[120:128, :],
                           in_=val[120:128, :]).then_inc(lsem, 16)
            sync.wait_ge(lsem, 32)
            sync.dma_start(out=out_qce[:, 0:C16A, :],
                           in_=in_bcast(0, 128, C16A)).then_inc(s1, 16)
            sync.dma_start(out=out_qce[0:120, C16A + C16B:NREP, :],
                           in_=in_bcast(0, 120, C15)).then_inc(s1, 16)
            sync.wait_ge(s1, 32)

        @block.scalar
        def _(scalar):
            scalar.wait_ge(lsem, 32)
            scalar.dma_start(out=out_qce[:, C16A:C16A + C16B, :],
                             in_=in_bcast(0, 128, C16B)).then_inc(s2, 16)
            scalar.dma_start(out=out_qce[120:128, C16A + C16B:NREP, :],
                             in_=in_bcast(120, 128, C15)).then_inc(s2, 16)
            scalar.wait_ge(s2, 32)

    return nc


def kernel(query=None, key=None, value=None, attn_mask=None, **_ignored):
    global LAST_RESULT
    value = np.ascontiguousarray(np.asarray(value, dtype=np.float32))
    vflat = value.reshape(16, F)                      # 16 chunks of 32KB
    vexp = np.ascontiguousarray(np.tile(vflat, (8, 1)))   # [128, F]

    nc = build_program()
    core_ids = list(range(N_CORES))
    in_maps = [{"value": vexp} for _ in core_ids]
    res = run_bass_kernel_spmd(nc, in_maps, core_ids, trace=TRACE,
                               **TRACE_KWARGS)
    LAST_RESULT = res

    # Core i supplies output rows [i*256, (i+1)*256).
    shards = [res.results[i]["out"].reshape(ROWS_PER_CORE, S, 1, D)
              for i in range(N_CORES)]
    return np.concatenate(shards, axis=0)


# revision 7
# speedup vs baseline: 1.1211x; 1.1211x over previous
"""Trainium2 Bass kernel for nn_Model_39676907882504.

Math: qk = (q @ k^T)/8 has shape [1,2048,1,1]; after the transposes it is
[2048,1,1,1], and softmax over the trailing size-1 axis is exactly 1.0
regardless of qk (exp(x-max)/sum == 1/1 bit-exactly). The final matmul
[S,Q,B,Q] @ [B,S,Q,D] with attn_weight == 1 therefore reduces to
broadcasting `value` across a new leading dim:

    output[i, j, 0, :] = value[0, j, 0, :]   for all i in [0, 2048)

i.e. a 512KB -> 1GiB broadcast copy.  Pure memory-regime kernel.

Sharding (per the hint): leading output dim (2048 rows) split across the
8 cores, 256 rows/core (= 4096 sub-rows of 32KB; sub-row d holds value
chunk d%16); value replicated.

Trace-derived facts this kernel is built around:

  * A dma_start's descriptors are split over SDMA engines contiguously
    along the OUTER AP dim, using the largest divisor <= 16 (128 -> 16
    engines x 8, 120 -> 15 engines x 8, 8 -> engines 0-7 x 1).
  * Each engine moves a 32KB descriptor in ~1216ns (its ~27 GB/s line
    rate); the kernel is engine-bound at ~435 GB/s aggregate.
  * SDMA engine 15 intermittently suffers ~2us stalls every ~8
    descriptors (~20% capacity loss, present in most runs, absent in
    some) — so engine 15 gets a reduced share sized for its degraded
    rate: 16-way stores cover 25 of the 32 c-reps; the other 7 go via a
    15-way [120, 7] store (engines 0-14) plus an [8, 7] tail (engines
    0-7) covering partitions 120-127.
  * Few instructions: per-instruction completion work also lands on
    engine 15 (~2.7us each), so the whole kernel is 2 loads + 4 stores.

SBUF tile: [128, 8192], partition q = chunk q%16 (8 copies of value,
host-pre-tiled).  Stores re-read it via a stride-0 middle dim; the DRAM
side is out.rearrange("(c q) e -> q c e"): descriptor (q, c) lands at
sub-row q + 128c, whose required content is chunk q%16.

Engine budget: engines 0-7 ~272 descs, 8-14 ~264, engine 15 200 descs
(sick: 200 x ~1.55us = 310us < 272 x 1.216us = 331us) -> ~331us span
+ ~13us fixed NEFF entry/exit, robust to engine 15's state.
"""

import sys

for _p in ("/opt/trn_rl_repo",):
    if _p not in sys.path:
        sys.path.insert(0, _p)

import numpy as np

import concourse.bass as bass
import concourse.mybir as mybir
from concourse.bass_utils import run_bass_kernel_spmd

S = 2048
D = 64
N_CORES = 8
ROWS_PER_CORE = S // N_CORES          # 256 output rows/core, 512KB each
F = 8192                              # f32 per 32KB chunk; value = 16 chunks
SUBROWS = ROWS_PER_CORE * 16          # 4096 32KB sub-rows per shard
NREP = SUBROWS // 128                 # 32 broadcast reps of the 128-part tile

C16A = 15                             # 16-way store c-reps on queue A
C16B = 10                             # 16-way store c-reps on queue B
C15 = NREP - C16A - C16B              # 7 reps routed around engine 15
# Queue A: loads + [128,C16A] + [8,C15] tail  = ~2104 descriptors
# Queue B: [128,C16B] + [120,C15] 15-way      = ~2120 descriptors

TRACE = False          # test.py flips this to profile
TRACE_KWARGS = {}
LAST_RESULT = None     # BassKernelResults of the last run (for test.py)


def build_program():
    nc = bass.Bass()
    val = nc.declare_dram_parameter("value", [128, F], mybir.dt.float32,
                                    isOutput=False)
    out = nc.declare_dram_parameter("out", [SUBROWS, F], mybir.dt.float32,
                                    isOutput=True)

    vtile = nc.alloc_sbuf_tensor("vtile", [128, F], mybir.dt.float32)

    # [q, c, e]: sub-row q + 128*c <- vtile partition q (chunk q%16).
    out_qce = out[:, :].rearrange("(c q) e -> q c e", q=128)

    def in_bcast(q0, q1, reps):
        return vtile[q0:q1, :].unsqueeze(1).broadcast_to((q1 - q0, reps, F))

    with nc.Block() as block, \
         nc.semaphore("lsem") as lsem, \
         nc.semaphore("s1") as s1, \
         nc.semaphore("s2") as s2:

        @block.sync
        def _(sync):
            # 15-way + 8-way loads keep engine 15 out of the load phase.
            sync.dma_start(out=vtile[0:120, :],
                           in_=val[0:120, :]).then_inc(lsem, 16)
            sync.dma_start(out=vtile[120:128, :],
                           in_=val[120:128, :]).then_inc(lsem, 16)
            sync.wait_ge(lsem, 32)
            sync.dma_start(out=out_qce[:, 0:C16A, :],
                           in_=in_bcast(0, 128, C16A)).then_inc(s1, 16)
            sync.dma_start(out=out_qce[120:128, C16A + C16B:NREP, :],
                           in_=in_bcast(120, 128, C15)).then_inc(s1, 16)
            sync.wait_ge(s1, 32)

        @block.scalar
        def _(scalar):
            scalar.wait_ge(lsem, 32)
            scalar.dma_start(out=out_qce[:, C16A:C16A + C16B, :],
                             in_=in_bcast(0, 128, C16B)).then_inc(s2, 16)
            scalar.dma_start(out=out_qce[0:120, C16A + C16B:NREP, :],
                             in_=in_bcast(0, 120, C15)).then_inc(s2, 16)
            scalar.wait_ge(s2, 32)

    return nc


def kernel(query=None, key=None, value=None, attn_mask=None, **_ignored):
    global LAST_RESULT
    value = np.ascontiguousarray(np.asarray(value, dtype=np.float32))
    vflat = value.reshape(16, F)                      # 16 chunks of 32KB
    vexp = np.ascontiguousarray(np.tile(vflat, (8, 1)))   # [128, F]

    nc = build_program()
    core_ids = list(range(N_CORES))
    in_maps = [{"value": vexp} for _ in core_ids]
    res = run_bass_kernel_spmd(nc, in_maps, core_ids, trace=TRACE,
                               **TRACE_KWARGS)
    LAST_RESULT = res

    # Core i supplies output rows [i*256, (i+1)*256).
    shards = [res.results[i]["out"].reshape(ROWS_PER_CORE, S, 1, D)
              for i in range(N_CORES)]
    return np.concatenate(shards, axis=0)
